# revision 28
# baseline (speedup 1.0000x reference)
"""Trainium2 Bass kernel for nn_EnsembleDynamicModel (baseline restore).

Ensemble MLP: E=7 members, x=[state(32)|action(8)] -> 256 -> 256 -> 256 -> 128
-> {mu(32), log_sigma(32)} with swish hidden activations, soft-clamped
log_sigma -> sigma=exp(.), and mu += state residual.

Strategy: data-parallel over the batch axis; feature-major activations.
"""

import os
import sys
import numpy as np
from contextlib import ExitStack

for _p in ("/opt/trn_rl_repo", "/root/.axon_site/_ro/trn_rl_repo"):
    if os.path.isdir(_p) and _p not in sys.path:
        sys.path.append(_p)

import ml_dtypes  # noqa: E402
import concourse.bass as bass  # noqa: E402
import concourse.tile as tile  # noqa: E402
import concourse.mybir as mybir  # noqa: E402
from concourse import bacc  # noqa: E402
from concourse.bass_utils import run_bass_kernel_spmd  # noqa: E402

F32 = mybir.dt.float32
AF = mybir.ActivationFunctionType
STORE = mybir.dt.bfloat16
NP_STORE = ml_dtypes.bfloat16
_mmv = lambda ap: ap

E = 7
B = 32768
S = 32
A = 8
DIN = S + A            # 40
NCORES = 8
BL = B // NCORES       # 4096 batch rows per core
CH = 2048              # batch chunk per psum tile (4 PSUM banks fp32)
NSUB = 512             # one matmul's free dim (1 PSUM bank fp32)
NCHUNK = BL // CH      # 2
NJ = CH // NSUB        # 4
NCONST = 8             # const columns per ensemble member


def _build_kernel(ctx, tc, io, act=AF.Silu):
    nc = tc.nc
    cpool = ctx.enter_context(tc.tile_pool(name="cpool", bufs=1))
    hpool = ctx.enter_context(tc.tile_pool(name="hpool", bufs=1))
    wpool = ctx.enter_context(tc.tile_pool(name="wpool", bufs=2))
    pspool = ctx.enter_context(tc.tile_pool(name="pspool", bufs=2, space="PSUM"))
    sgpool = ctx.enter_context(tc.tile_pool(name="sgpool", bufs=3))

    def load_weights(e, first=False):
        w0 = wpool.tile([DIN, 256], STORE, tag="w0", name="w0")
        if first:
            nc.sync.dma_start(xt[:, 0:CH], io["xt"][:, 0:CH])
            nc.sync.dma_start(w0[:], io["w0"][e])
            nc.sync.dma_start(cns[:], io["cns"])
            nc.sync.dma_start(sgc[:], io["sgc"])
            nc.sync.dma_start(xt[:, CH:BL], io["xt"][:, CH:BL])
        else:
            nc.sync.dma_start(w0[:], io["w0"][e])
        w1, w2, w3 = [], [], []
        for k in range(2):
            t = wpool.tile([128, 256], STORE, tag=f"w1_{k}", name=f"w1_{k}")
            nc.sync.dma_start(_mmv(t[:]),
                              _mmv(io["w1"][e, k * 128:(k + 1) * 128, :]))
            w1.append(t)
            t = wpool.tile([128, 256], STORE, tag=f"w2_{k}", name=f"w2_{k}")
            nc.sync.dma_start(_mmv(t[:]),
                              _mmv(io["w2"][e, k * 128:(k + 1) * 128, :]))
            w2.append(t)
            t = wpool.tile([128, 128], STORE, tag=f"w3_{k}", name=f"w3_{k}")
            nc.sync.dma_start(_mmv(t[:]),
                              _mmv(io["w3"][e, k * 128:(k + 1) * 128, :]))
            w3.append(t)
        wh = wpool.tile([128, 64], STORE, tag="wh", name="wh")
        nc.sync.dma_start(_mmv(wh[:]), _mmv(io["wh"][e]))
        if first:
            nc.sync.dma_start(resid[:], io["resid"])
        return w0, w1, w2, w3, wh

    scratch = cpool.tile([1, 8], F32, tag="scratch")
    nc.gpsimd.memset(scratch[:], 0.0)
    nc.scalar.activation(scratch[0:1, 0:8], scratch[0:1, 0:8], act, bias=0.0)

    xt = cpool.tile([DIN, BL], STORE, tag="xt")
    cns = cpool.tile([128, E * NCONST], F32, tag="cns")
    sgc = cpool.tile([128, 2], F32, tag="sgc")
    resid = cpool.tile([64, BL], F32, tag="resid")

    pk = [sgpool.tile([128, BL], F32, tag=f"pk{g}", name=f"pk{g}", bufs=1)
          for g in range(2)]

    hA = [hpool.tile([128, BL], STORE, tag=f"hA{i}", name=f"hA{i}")
          for i in range(2)]
    hB = [hpool.tile([128, BL], STORE, tag=f"hB{i}", name=f"hB{i}")
          for i in range(2)]

    def gemm_layer(parts, m_tiles, h_out, bias_cols, e, chunks=None):
        """parts: list of (h_tile, h_rows, w_tile, w_rows) k-passes."""
        nkt = len(parts)
        for c in (range(NCHUNK) if chunks is None else chunks):
            for mt in range(m_tiles):
                ps = pspool.tile([128, CH], F32, tag="ps", name="ps")
                for kt, (ht, hr, wt, wr) in enumerate(parts):
                    for j in range(NJ):
                        ncol = slice(c * CH + j * NSUB, c * CH + (j + 1) * NSUB)
                        nc.tensor.matmul(
                            ps[:, j * NSUB:(j + 1) * NSUB],
                            wt[wr, mt * 128:(mt + 1) * 128],
                            ht[hr, ncol],
                            start=(kt == 0),
                            stop=(kt == nkt - 1),
                            skip_group_check=True,
                        )
                bcol = e * NCONST + bias_cols[mt]
                nc.scalar.activation(
                    _mmv(h_out[mt][:, c * CH:(c + 1) * CH]), ps[:, :],
                    act, bias=cns[:, bcol:bcol + 1],
                )

    def l0_parts(w0):
        return [(xt, slice(0, DIN), w0, slice(0, DIN))]

    def whole(h_list, w_list):
        return [(h_list[k], slice(0, 128), w_list[k], slice(0, 128))
                for k in range(len(w_list))]

    w_cur = None
    for e in range(E):
        if e == 0:
            w_cur = load_weights(0, first=True)
            gemm_layer(l0_parts(w_cur[0]), 2, hA, (0, 1), 0)   # L0 of e=0
        w0, w1, w2, w3, wh = w_cur

        gemm_layer(whole(hA, w1), 2, hB, (2, 3), e)    # 256  -> 256
        gemm_layer(whole(hB, w2), 2, hA, (4, 5), e)    # 256  -> 256
        # last member: L3 chunk 1 is emitted piecewise in the tail below
        gemm_layer(whole(hA, w3), 1, hB, (6,), e,
                   chunks=[0] if e == E - 1 else None)  # 256 -> 128 (hB[0])
        h3 = hB[0]

        def head_chunk(base, width, nsplit):
            cs = slice(base, base + width)
            ps = pspool.tile([64, width], F32, tag="ps", name="psh")
            for j in range(width // NSUB):
                ncol = slice(base + j * NSUB, base + (j + 1) * NSUB)
                nc.tensor.matmul(
                    ps[:, j * NSUB:(j + 1) * NSUB],
                    _mmv(wh[:, :]), _mmv(h3[:, ncol]),
                    start=True, stop=True,
                )
            bcol = e * NCONST + 7
            hd = sgpool.tile([64, width], F32, tag="hd", name="hd")
            nc.vector.affine_then_add(
                hd[:, :], ps[:, :], resid[:, cs], 1.0,
                cns[0:64, bcol:bcol + 1],
            )
            step = width // nsplit
            for p in range(nsplit):
                pcs = slice(base + p * step, base + (p + 1) * step)
                pls = slice(p * step, (p + 1) * step)
                nc.sync.dma_start(io["mu"][e * 32:(e + 1) * 32, pcs],
                                  hd[0:32, pls])
            g, r = divmod(e, 4)
            if e == E - 1:
                sg2 = sgpool.tile([64, width], F32, tag="sg2e", name="sg2e",
                                  bufs=2)
                nc.scalar.activation(sg2[32:64, :], hd[32:64, :], AF.Tanh,
                                     scale=0.5)
                sg3 = sgpool.tile([64, width], F32, tag="sg3e", name="sg3e",
                                  bufs=2)
                nc.vector.tensor_scalar(
                    sg3[32:64, :], sg2[32:64, :],
                    sgc[32:64, 0:1], sgc[32:64, 1:2],
                    mybir.AluOpType.mult, mybir.AluOpType.add,
                )
                for p in range(width // NSUB):
                    pcs = slice(base + p * NSUB, base + (p + 1) * NSUB)
                    pls = slice(p * NSUB, (p + 1) * NSUB)
                    nc.sync.dma_start(io["sig"][e * 32:(e + 1) * 32, pcs],
                                      sg3[32:64, pls])
            else:
                nc.vector.tensor_copy(pk[g][r * 32:(r + 1) * 32, cs],
                                      hd[32:64, :])

        def flush(base, width):
            """Packed sigma tanh -> scale -> DMA for a column range.

            Emitted mid-seam so the tanh (SBUF-sourced, PE-independent)
            absorbs the ACT stall while the cold PE grinds through the
            head + L0 + L1 chain of the next member.
            """
            rows = 128 if e == 3 else 64
            g = e // 4
            cs = slice(base, base + width)
            sg2 = sgpool.tile([128, width], F32, tag="sg2", name="sg2",
                              bufs=2)
            nc.scalar.activation(sg2[0:rows, :], pk[g][0:rows, cs],
                                 AF.Tanh, scale=0.5)
            sg3 = sgpool.tile([128, width], F32, tag="sg3", name="sg3",
                              bufs=2)
            nc.vector.tensor_scalar(
                sg3[0:rows, :], sg2[0:rows, :],
                sgc[0:rows, 0:1], sgc[0:rows, 1:2],
                mybir.AluOpType.mult, mybir.AluOpType.add,
            )
            nc.sync.dma_start(io["sig"][g * 128:g * 128 + rows, cs],
                              sg3[0:rows, :])

        if e == E - 1:
            # tail: heads for cols 0:2048 overlap L3 c1's matmuls; L3 c1 is
            # drained in 3 column pieces (j-outer matmuls) so the final
            # 512-wide affine->tanh->scale->DMA chains unlock early
            head_chunk(0, 1024, 1)
            head_chunk(1024, 1024, 1)
            ps3 = pspool.tile([128, CH], F32, tag="ps", name="ps")
            for j in range(NJ):
                ncol = slice(CH + j * NSUB, CH + (j + 1) * NSUB)
                for kt in range(2):
                    nc.tensor.matmul(
                        ps3[:, j * NSUB:(j + 1) * NSUB],
                        w3[kt][0:128, 0:128],
                        hA[kt][0:128, ncol],
                        start=(kt == 0), stop=(kt == 1),
                        skip_group_check=True,
                    )
            bcol3 = e * NCONST + 6
            for p0, pw, hw in ((0, 1024, 1024), (1024, 512, 512),
                               (1536, 512, 512)):
                nc.scalar.activation(
                    hB[0][:, CH + p0:CH + p0 + pw], ps3[:, p0:p0 + pw],
                    act, bias=cns[:, bcol3:bcol3 + 1],
                )
                head_chunk(CH + p0, hw, 1)
        else:
            head_chunk(0, CH, 1)
            w_cur = load_weights(e + 1)
            gemm_layer(l0_parts(w_cur[0]), 2, hA, (0, 1), e + 1)  # L0 of e+1
            if e in (3, 5):
                flush(0, CH)        # chunk-0 rows packed; fills the seam gap
            head_chunk(CH, CH, 1)
            if e in (3, 5):
                flush(CH, CH)       # chunk-1 rows packed just above


def build_program(act=AF.Silu):
    nc = bacc.Bacc(
        "TRN2", target_bir_lowering=False, debug=False, num_devices=NCORES
    )
    io = {
        "xt": nc.dram_tensor("xt", [DIN, BL], STORE,
                             kind="ExternalInput").ap(),
        "resid": nc.dram_tensor("resid", [64, BL], F32,
                                kind="ExternalInput").ap(),
        "w0": nc.dram_tensor("w0", [E, DIN, 256], STORE,
                             kind="ExternalInput").ap(),
        "w1": nc.dram_tensor("w1", [E, 256, 256], STORE,
                             kind="ExternalInput").ap(),
        "w2": nc.dram_tensor("w2", [E, 256, 256], STORE,
                             kind="ExternalInput").ap(),
        "w3": nc.dram_tensor("w3", [E, 256, 128], STORE,
                             kind="ExternalInput").ap(),
        "wh": nc.dram_tensor("wh", [E, 128, 64], STORE,
                             kind="ExternalInput").ap(),
        "cns": nc.dram_tensor("cns", [128, E * NCONST], F32,
                              kind="ExternalInput").ap(),
        "sgc": nc.dram_tensor("sgc", [128, 2], F32, kind="ExternalInput").ap(),
        "mu": nc.dram_tensor("mu", [E * 32, BL], F32,
                             kind="ExternalOutput").ap(),
        "sig": nc.dram_tensor("sig", [E * 32, BL], F32,
                              kind="ExternalOutput").ap(),
    }
    with tile.TileContext(nc) as tc, ExitStack() as ctx:
        _build_kernel(ctx, tc, io, act=act)
    nc.compile()
    return nc


def host_prep(state, action, W0, b0, W1, b1, W2, b2, W3, b3,
              Wmu, bmu, Wsig, bsig, max_logstd, min_logstd):
    f = lambda a: np.ascontiguousarray(np.asarray(a), dtype=np.float32)
    g = lambda a: np.ascontiguousarray(np.asarray(a, dtype=np.float32)
                                       .astype(NP_STORE))
    state, action = f(state), f(action)
    xt_full = np.ascontiguousarray(
        np.concatenate([state, action], axis=1).T
    )  # [40, B] fp32
    wh = np.concatenate([f(Wmu), f(Wsig)], axis=2)
    b0, b1, b2, b3 = f(b0), f(b1), f(b2), f(b3)
    bmu, bsig = f(bmu), f(bsig)
    mx, mn = f(max_logstd), f(min_logstd)

    cns = np.zeros((128, E * NCONST), np.float32)
    for e in range(E):
        c = e * NCONST
        cns[:, c + 0] = b0[e, :128]
        cns[:, c + 1] = b0[e, 128:]
        cns[:, c + 2] = b1[e, :128]
        cns[:, c + 3] = b1[e, 128:]
        cns[:, c + 4] = b2[e, :128]
        cns[:, c + 5] = b2[e, 128:]
        cns[:, c + 6] = b3[e, :]
        cns[0:32, c + 7] = bmu[e]
        cns[32:64, c + 7] = bsig[e] - mx   # sigma-head drain bias
    sgc = np.zeros((128, 2), np.float32)
    sgc[:, 0] = np.tile(np.exp(mx) / 2, 4)
    sgc[:, 1] = np.tile(np.exp(mn) + np.exp(mx) / 2, 4)

    shared = {
        "w0": g(W0), "w1": g(W1), "w2": g(W2), "w3": g(W3), "wh": g(wh),
        "cns": cns, "sgc": sgc,
    }
    resid_full = np.zeros((64, B), np.float32)
    resid_full[0:32] = xt_full[0:32]
    xt_store = xt_full.astype(NP_STORE)
    shards = [
        {
            "xt": np.ascontiguousarray(xt_store[:, c * BL:(c + 1) * BL]),
            "resid": np.ascontiguousarray(resid_full[:, c * BL:(c + 1) * BL]),
        }
        for c in range(NCORES)
    ]
    return shared, shards


def host_post(results):
    mu = np.empty((E, B, 32), np.float32)
    sigma = np.empty((E, B, 32), np.float32)
    for c in range(NCORES):
        bs = slice(c * BL, (c + 1) * BL)
        mu[:, bs, :] = results[c]["mu"].reshape(E, 32, BL).transpose(0, 2, 1)
        sigma[:, bs, :] = results[c]["sig"].reshape(E, 32, BL).transpose(0, 2, 1)
    return mu, sigma


_PROGRAM = None


def _get_program():
    global _PROGRAM
    if _PROGRAM is None:
        _PROGRAM = build_program()
    return _PROGRAM


def kernel(**inputs):
    nc = _get_program()
    shared, shards = host_prep(**inputs)
    in_maps = [{**shared, **shards[c]} for c in range(NCORES)]
    res = run_bass_kernel_spmd(nc, in_maps, list(range(NCORES)))
    return host_post(res.results)
